# revision 28
# baseline (speedup 1.0000x reference)
"""Trainium2 Bass kernel for windowed multi-head attention with dynamic
position bias (sparse_attention, B=2, H=W=256, 8x32 windows, 6 heads, d=32).

Strategy (data-parallel over windows, 8 cores x 64 windows):
  host:   im2win + shard + pre-transpose Q,K to [c, token] fp16 layout (scale
          folded into Q), evaluate the tiny pos-bias MLP + gather to the 6
          per-head (256,256) bias matrices; all arrays window-major so DMA
          runs are 4-8KB per partition.  Normalization (1/rowsum) on host.
  device, per window (software-pipelined; PV stage runs one window behind):
    - one 6-bank PSUM tile holds S^T for all 6 heads (head h at cols 512h)
    - heads 0,1,2: S^T alone (start=stop QK matmuls, contract 32, row-tiled
      fp16); softmax exp via Schraudolph bit-trick in ONE VectorE
      scalar_tensor_tensor: i16(S*1477.32 + (15320 + 1477.32*bias)) -- the
      int16 bits ARE fp16 exp(S+bias) (+-3% per element, bias free).
    - heads 3,4,5: additive bias streamed into PSUM via fp16 identity
      matmuls, then S^T += K^T.T Q^T, then ONE exact ScalarE exp.
    - P V with P^T chunks as stationaries and ones-augmented V as moving
      operand -> O lands [q, (h,d)+rowsum] in one PSUM bank; ONE ScalarE
      copy moves raw O+rowsum to fp16 output slabs; host divides.
"""

import math
import sys

sys.path.insert(0, "/opt/trn_rl_repo")

import numpy as np

import concourse.bass as bass
import concourse.tile as tile
from concourse import mybir
from concourse.alu_op_type import AluOpType
from concourse.bass_utils import run_bass_kernel_spmd

F32 = mybir.dt.float32
FP16 = mybir.dt.float16
I16 = mybir.dt.int16
EXP = mybir.ActivationFunctionType.Exp

N_CORES = 8
B, H, W = 2, 256, 256
H_SP, W_SP = 8, 32
NUM_HEADS = 6
DIM = 192
HEAD_DIM = 32
SCALE = HEAD_DIM ** -0.5
N = H_SP * W_SP                     # 256 tokens / window
NW_TOTAL = B * (H // H_SP) * (W // W_SP)   # 512 windows
NW = NW_TOTAL // N_CORES            # 64 windows / core

SCHR_HEADS = (0, 1, 2)              # exp via DVE Schraudolph (bias fused)
SCALAR_HEADS = (3, 4, 5)            # exact exp on ScalarE (bias via fp16 PE)
# S^T PSUM regions per window (three 2-bank tiles): R0 = heads 0,1 (DVE,
# early STT), R1 = head 3 (ScalarE, early exp) + head 2 (DVE, late STT),
# R2 = heads 4,5 (ScalarE).  The schedule shapes the PSUM-reuse chains so
# each region's exp ends well before window w+1 re-touches it.
S_SLOT = {0: (0, 0), 1: (0, 512), 3: (1, 0), 2: (1, 512),
          4: (2, 0), 5: (2, 512)}
# P (softmax numerator) SBUF column base per head, grouped by writer so
# each exp/STT instruction writes one contiguous range.
PT_BASE = {0: 0, 1: 512, 2: 1024, 3: 1536, 4: 2048, 5: 2560}
A16 = 1024.0 / math.log(2.0)        # Schraudolph scale for fp16 bits
B16 = 15360.0 - 40.0                # exponent offset - error-centering shift

WG = 8     # windows per input slab (4KB/partition DMA runs)
OG = 4     # windows per output slab


# --------------------------------------------------------------------------
# device program
# --------------------------------------------------------------------------
def build_program(nw=NW):
    from concourse import bacc
    nc = bacc.Bacc("TRN2", target_bir_lowering=False, debug=False)

    qT = nc.dram_tensor("qT", [DIM, nw * N], FP16, kind="ExternalInput").ap()
    kT = nc.dram_tensor("kT", [DIM, nw * N], FP16, kind="ExternalInput").ap()
    vA = nc.dram_tensor("vA", [128, nw * 396], FP16, kind="ExternalInput").ap()
    biasT = nc.dram_tensor("biasT", [3, 128, 512], FP16,
                           kind="ExternalInput").ap()
    ident = nc.dram_tensor("ident", [128, 128], FP16,
                           kind="ExternalInput").ap()
    bmat = nc.dram_tensor("bmat", [128, 3 * 512], F32,
                          kind="ExternalInput").ap()
    outw = nc.dram_tensor("outw", [128, nw * 396], FP16,
                          kind="ExternalOutput").ap()

    with tile.TileContext(nc) as tc:
        _emit(nc, tc, nw, qT, kT, vA, biasT, ident, bmat, outw)
    nc.compile()
    return nc


def _emit(nc, tc, nw, qT, kT, vA, biasT, ident, bmat, outw):
    from contextlib import ExitStack
    ctx = ExitStack()

    # resident constants
    bias_sb = nc.alloc_sbuf_tensor("bias_sb", [128, 3 * 512], FP16).ap()
    id_sb = nc.alloc_sbuf_tensor("id_sb", [128, 128], FP16).ap()
    bmat_sb = nc.alloc_sbuf_tensor("bmat_sb", [128, 3 * 512], F32).ap()
    nc.sync.dma_start(
        bias_sb.rearrange("p (h f) -> p h f", h=3),
        biasT.rearrange("h p f -> p h f"),
    )
    nc.sync.dma_start(id_sb, ident)
    nc.sync.dma_start(bmat_sb, bmat)

    pin = ctx.enter_context(tc.tile_pool(name="pin", bufs=2))
    pps = ctx.enter_context(tc.tile_pool(name="pps", bufs=2, space="PSUM"))
    ppt = ctx.enter_context(tc.tile_pool(name="ppt", bufs=3))
    pout = ctx.enter_context(tc.tile_pool(name="pout", bufs=4))

    qa = qb = ka = kb = va = ob = None
    pend = None      # (pt, va, wv, w) of the previous window, PV pending

    def emit_pv(nc, state):
        pt, pva, pwv, pw = state
        pv = pps.tile([128, 396], F32, tag="pv", bufs=2, name="pv")
        for qc in (0, 1):
            for h in range(NUM_HEADS):
                for kc in (0, 1):
                    nc.tensor.matmul(
                        pv[:, 198 * qc + 33 * h: 198 * qc + 33 * h + 33],
                        lhsT=pt[:, PT_BASE[h] + 256 * kc + 128 * qc:
                                PT_BASE[h] + 256 * kc + 128 * qc + 128],
                        rhs=pva[:, pwv + 198 * kc + 33 * h:
                                pwv + 198 * kc + 33 * h + 33],
                        start=(kc == 0), stop=(kc == 1),
                        skip_group_check=True,
                    )
        # raw O + rowsums -> fp16 output slab; host divides
        nc.scalar.copy(ob[:, (pw % OG) * 396: (pw % OG) * 396 + 396], pv)
        if pw % OG == OG - 1:  # flush the output slab in one DMA
            nc.sync.dma_start(
                outw[:, (pw - (OG - 1)) * 396: (pw + 1) * 396], ob)

    # slab schedule: small first slab so compute starts early, then WG-sized
    slabs = [(0, 2), (2, WG - 2)] + [(s, WG) for s in range(WG, nw, WG)]
    slab_of = {}
    for s0, sn in slabs:
        for i in range(sn):
            slab_of[s0 + i] = s0

    for w in range(nw):
        if w in (s[0] for s in slabs) and slab_of[w] == w:
            sn = dict(slabs)[w]
            g = w * N
            qa = pin.tile([128, WG * N], FP16, tag="qa",
                          padded_shape=[128, WG * N])
            nc.sync.dma_start(qa[:, 0:sn * N], qT[0:128, g:g + sn * N])
            qb = pin.tile([64, WG * N], FP16, tag="qb",
                          padded_shape=[64, WG * N])
            nc.sync.dma_start(qb[:, 0:sn * N], qT[128:192, g:g + sn * N])
            ka = pin.tile([128, WG * N], FP16, tag="ka",
                          padded_shape=[128, WG * N])
            nc.sync.dma_start(ka[:, 0:sn * N], kT[0:128, g:g + sn * N])
            kb = pin.tile([64, WG * N], FP16, tag="kb",
                          padded_shape=[64, WG * N])
            nc.sync.dma_start(kb[:, 0:sn * N], kT[128:192, g:g + sn * N])
            va = pin.tile([128, WG * 396], FP16, tag="va",
                          padded_shape=[128, WG * 396])
            nc.sync.dma_start(va[:, 0:sn * 396],
                              vA[:, w * 396:(w + sn) * 396])
        wq = (w - slab_of[w]) * N  # this window's offset in the input slabs

        sr = [pps.tile([128, 1024], F32, tag="s", bufs=3, name=f"s{i}")
              for i in range(3)]
        pt = ppt.tile([128, 3072], FP16, tag="pt")

        def qk1(h, kc, schr):
            grp, base = S_SLOT[h]
            hp = h if h < 4 else h - 4
            ktile = ka if h < 4 else kb
            qtile = qa if h < 4 else qb
            nc.tensor.matmul(
                sr[grp][:, base + 256 * kc: base + 256 * kc + 256],
                lhsT=ktile[32 * hp: 32 * hp + 32,
                           wq + 128 * kc: wq + 128 * kc + 128],
                rhs=qtile[32 * hp: 32 * hp + 32, wq: wq + N],
                start=schr, stop=(schr or kc == 1),
                tile_position=(32 * hp, 0), skip_group_check=True,
            )

        def bias(h):
            grp, base = S_SLOT[h]
            j = SCALAR_HEADS.index(h)
            nc.tensor.matmul(
                sr[grp][:, base: base + 512], lhsT=id_sb,
                rhs=bias_sb[:, 512 * j: 512 * j + 512],
                start=True, stop=False, skip_group_check=True,
            )

        # Phase 1: head 3's bias, then a 3-position QK rotation (h3, h0, h1)
        # so adjacent matmuls overlap in the PE array.  exp(h3) starts very
        # early; STT-R0 follows as soon as h0/h1 land.
        bias(3)
        qk1(3, 0, False); qk1(0, 0, True); qk1(1, 0, True)
        qk1(3, 1, False)
        nc.scalar.activation(pt[:, 1536:2048], sr[1][:, 0:512], EXP)
        qk1(0, 1, True); qk1(1, 1, True)
        nc.vector.scalar_tensor_tensor(
            pt[:, 0:1024].bitcast(I16), sr[0][:, 0:1024], A16,
            bmat_sb[:, 0:1024], op0=AluOpType.mult, op1=AluOpType.add)
        # Phase 2: heads 4,5 biases + QK rotation (h4, h5, h2).  h2 (DVE) is
        # late in the stream so its region is re-touched latest next window,
        # hiding STT-R1's completion.
        bias(4)
        bias(5)
        qk1(4, 0, False); qk1(5, 0, False); qk1(2, 0, True)
        qk1(4, 1, False); qk1(5, 1, False)
        nc.scalar.activation(pt[:, 2048:3072], sr[2][:, 0:1024], EXP)
        qk1(2, 1, True)
        nc.vector.scalar_tensor_tensor(
            pt[:, 1024:1536].bitcast(I16), sr[1][:, 512:1024], A16,
            bmat_sb[:, 1024:1536], op0=AluOpType.mult, op1=AluOpType.add)

        # PV + output copy run one window behind, so PE never stalls on exp.
        if pend is not None:
            emit_pv(nc, pend)
        if w % OG == 0:   # fresh output slab for THIS window's deferred PV
            ob = pout.tile([128, OG * 396], FP16, tag="ob", bufs=3)
        pend = (pt, va, (w - slab_of[w]) * 396, w)

    emit_pv(nc, pend)
    ctx.close()


# --------------------------------------------------------------------------
# host side
# --------------------------------------------------------------------------
def _layer_norm(x, g, b, eps=1e-5):
    m = x.mean(-1, keepdims=True)
    v = x.var(-1, keepdims=True)
    return (x - m) / np.sqrt(v + eps) * g + b


def compute_bias(rpe_biases, rel_index, pos_proj_w, pos_proj_b, ln1_g, ln1_b,
                 fc1_w, fc1_b, ln2_g, ln2_b, fc2_w, fc2_b, ln3_g, ln3_b,
                 fc3_w, fc3_b):
    """pos-bias MLP + gather, in fp64 on host -> (6, 256, 256) fp32 [h, q, k]."""
    f8 = np.float64
    p = rpe_biases.astype(f8) @ pos_proj_w.astype(f8) + pos_proj_b.astype(f8)
    p = np.maximum(_layer_norm(p, ln1_g.astype(f8), ln1_b.astype(f8)), 0)
    p = p @ fc1_w.astype(f8) + fc1_b.astype(f8)
    p = np.maximum(_layer_norm(p, ln2_g.astype(f8), ln2_b.astype(f8)), 0)
    p = p @ fc2_w.astype(f8) + fc2_b.astype(f8)
    p = np.maximum(_layer_norm(p, ln3_g.astype(f8), ln3_b.astype(f8)), 0)
    pos = p @ fc3_w.astype(f8) + fc3_b.astype(f8)          # (num_biases, 6)
    rel = pos[np.asarray(rel_index).reshape(-1)]
    return np.ascontiguousarray(
        rel.reshape(N, N, NUM_HEADS).transpose(2, 0, 1)).astype(np.float32)


def im2win(x):
    """(B, L, C) -> (512, 256, C) windows in (b, hb, wb) / (hs, ws) order."""
    x = x.reshape(B, H // H_SP, H_SP, W // W_SP, W_SP, DIM)
    x = x.transpose(0, 1, 3, 2, 4, 5)
    return np.ascontiguousarray(x.reshape(NW_TOTAL, N, DIM))


def prep_consts(bias):
    """Device constants from the (6, 256, 256) fp32 bias [h, q, k]."""
    # biasT[j][k_local, 256*kc + q] = bias[h][q, 128*kc + k_local]
    bt = np.zeros((3, 128, 512), np.float32)
    for j, h in enumerate(SCALAR_HEADS):
        b = bias[h].transpose(1, 0).reshape(2, 128, N)     # [kc, k, q]
        bt[j] = b.transpose(1, 0, 2).reshape(128, 512)
    biasT = bt.astype(np.float16)
    identity = np.eye(128, dtype=np.float32).astype(np.float16)

    # bmat[k, 512*jj + 256*kc + q] = B16 + A16*bias[h][q, 128kc+k]
    bmat = np.zeros((128, 3 * 512), np.float32)
    for jj, h in enumerate(SCHR_HEADS):
        b = bias[h].transpose(1, 0).reshape(2, 128, N)     # [kc, k, q]
        bmat[:, 512 * jj: 512 * (jj + 1)] = (
            np.float32(B16) + np.float32(A16)
            * b.transpose(1, 0, 2).reshape(128, 512))
    return biasT, identity, bmat


def prep_inputs(qkv):
    """Build the full (unsharded) device arrays in window-major layouts so
    per-partition DMA runs span many windows. Shard by slicing axis 1."""
    q = im2win(np.asarray(qkv[0]))
    k = im2win(np.asarray(qkv[1]))
    v = im2win(np.asarray(qkv[2]))

    # (512, 256, 192) -> [192, 512, 256]: partition-major, windows inner
    qTf = np.ascontiguousarray(
        (q * np.float32(SCALE)).transpose(2, 0, 1)).astype(np.float16)
    kTf = np.ascontiguousarray(k.transpose(2, 0, 1)).astype(np.float16)

    vr = v.reshape(NW_TOTAL, 2, 128, NUM_HEADS, HEAD_DIM)
    ones = np.ones((NW_TOTAL, 2, 128, NUM_HEADS, 1), np.float32)
    vAf = np.concatenate([vr, ones], -1)          # (512, 2, 128, 6, 33)
    vAf = np.ascontiguousarray(
        vAf.reshape(NW_TOTAL, 2, 128, 198).transpose(2, 0, 1, 3)
    ).reshape(128, NW_TOTAL, 396).astype(np.float16)
    return qTf, kTf, vAf


def _run(qkv, rpe_biases, rel_index, params, trace=False, **spmd_kwargs):
    qkv = np.asarray(qkv, np.float32)
    bias = compute_bias(np.asarray(rpe_biases), np.asarray(rel_index), **params)
    qTf, kTf, vAf = prep_inputs(qkv)
    biasT, identity, bmat = prep_consts(bias)

    nc = build_program(NW)
    in_maps = []
    for c in range(N_CORES):
        s = slice(c * NW, (c + 1) * NW)
        in_maps.append({
            "qT": np.ascontiguousarray(qTf[:, s]).reshape(DIM, NW * N),
            "kT": np.ascontiguousarray(kTf[:, s]).reshape(DIM, NW * N),
            "vA": np.ascontiguousarray(vAf[:, s]).reshape(128, NW * 396),
            "biasT": biasT, "ident": identity, "bmat": bmat,
        })
    res = run_bass_kernel_spmd(nc, in_maps, core_ids=list(range(N_CORES)),
                               trace=trace, **spmd_kwargs)

    outw = np.stack([res.results[c]["outw"] for c in range(N_CORES)])
    # outw: (8, 128, NW*2*198) fp16 raw O + rowsums -> normalize on host
    x = outw.reshape(N_CORES, 128, NW, 2, 6, 33).astype(np.float32)
    o = x[..., 0:32]                              # (8, 128, nw, 2, 6, 32)
    rs = x[..., 32:33]
    xn = (o / rs).transpose(0, 2, 3, 1, 4, 5)     # (8, nw, 2, 128, 6, 32)
    xn = xn.reshape(NW_TOTAL, N, DIM)
    return unwindow(np.ascontiguousarray(xn)), res


def kernel(qkv, H=None, W=None, rpe_biases=None, rel_index=None, **params):
    return _run(qkv, rpe_biases, rel_index, params)[0]


def unwindow(x):
    """(512, 256, 192) -> (B, H, W, C)"""
    x = x.reshape(B, H // H_SP, W // W_SP, H_SP, W_SP, DIM)
    x = x.transpose(0, 1, 3, 2, 4, 5)
    return np.ascontiguousarray(x.reshape(B, H, W, DIM))
